# revision 6
# baseline (speedup 1.0000x reference)
"""Multi-head causal attention (dense_transformer) on 8 trn2 NeuronCores.

Problem: x[4, 2048, 768], 12 heads of d_head=64, causal softmax, out proj.

Sharding: data-parallel over batch (4) x tensor-parallel over heads
(2 groups of 6). Core c handles (batch c//2, heads 6*(c%2)..6*(c%2)+5) and
returns its partial output sum over its heads; the host adds the two
partials per batch ("all-reduce" of size 2 done host-side).

v3 layout/schedule (vs v1):
  - The schedule interleaves projection / output-projection matmuls as
    "ballast" into the exp-bound attention phase so the PE never idles
    while ACT chews through the 120 exps: chunk j's attention is paced
    against chunk j+1's projections (j=3 against the deferred output
    projections of chunks 0-2), spread proportionally over the steps.
    Chunk-j projections always precede chunk-j attention in every engine
    queue (queues execute in order; the reverse would deadlock).
  - Scores use the K=128 zero-padded form (KT pair-packed, QTz per head):
    row-tiled K=64 concurrent pairs measured SERIAL (~297ns/MM vs 241)
    on this HW path, so zero-padding the contraction is faster.
  - xT is DMA'd q-chunk 0 first so the first projections start as soon
    as possible; weights stream next, then the rest of x.
  - PSUM: scores pool 2x[128,1024] (4 banks) + z pool 2x[65,512]
    (2 banks) + proj pool 2x[128,512] (2 banks) = 8 banks.
  - exp: softmax without max-subtraction (scores are O(1) here; exp
    cannot overflow): P = exp(s/8), denominator accumulated via a
    constant-1 65th column of V in the PV matmul. Normalization:
    reciprocal read straight from PSUM -> DRAM hop -> partition-broadcast
    DMA -> multiply into zT. All PSUM->SBUF copies run on DVE, keeping
    ACT exclusively for exp.

Biases: b_K cancels in softmax; b_V/b_O fold into a constant row added
host-side; b_Q is always zero (falls back to numpy otherwise).
"""
import os
import sys
from collections import deque

sys.path.insert(0, "/opt/trn_rl_repo")

import numpy as np
import ml_dtypes

D_MODEL, N_HEADS, D_HEAD = 768, 12, 64
BATCH, SEQ = 4, 2048
HPG = 6           # heads per group (per core)
NPAIR = HPG // 2  # head pairs per core
NCORES = 8
QC = 512          # q chunk (moving operand width)
KT_TILES = SEQ // 128
QC_TILES = SEQ // QC
MT = D_MODEL // 128  # contraction tiles for projections
BF16 = ml_dtypes.bfloat16
FAR = 10 ** 9     # "no deadline" for ballast items

_prog_cache = {}


def _numpy_ref(normalized_resid_pre, W_Q, W_K, W_V, W_O, b_Q, b_K, b_V, b_O):
    x = normalized_resid_pre.astype(np.float32)
    Q = np.einsum("bsm,hmd->bshd", x, W_Q) + b_Q
    K = np.einsum("bsm,hmd->bshd", x, W_K) + b_K
    V = np.einsum("bsm,hmd->bshd", x, W_V) + b_V
    scores = np.einsum("bqhd,bkhd->bhqk", Q, K) / np.sqrt(np.float32(W_Q.shape[-1]))
    s = x.shape[1]
    causal = np.tril(np.ones((s, s), dtype=bool))
    scores = np.where(causal, scores, -np.inf)
    scores -= scores.max(axis=-1, keepdims=True)
    e = np.exp(scores)
    probs = e / e.sum(axis=-1, keepdims=True)
    z = np.einsum("bkhd,bhqk->bqhd", V, probs)
    return (np.einsum("bqhd,hdm->bqm", z, W_O) + b_O).astype(np.float32)


def _build_program():
    from concourse import bacc, tile
    import concourse.bass as bass
    import concourse.mybir as mybir

    f32 = mybir.dt.float32
    bf16 = mybir.dt.bfloat16

    nc = bacc.Bacc(None)
    xT_d = nc.dram_tensor("xT", [D_MODEL, SEQ], bf16, kind="ExternalInput")
    wq_d = nc.dram_tensor("wq", [D_MODEL, HPG * D_HEAD], bf16, kind="ExternalInput")
    wk_d = nc.dram_tensor("wk", [D_MODEL, HPG * D_HEAD], bf16, kind="ExternalInput")
    wv_d = nc.dram_tensor("wv", [D_MODEL, HPG * D_HEAD], bf16, kind="ExternalInput")
    wo_d = nc.dram_tensor("wo", [HPG * D_HEAD, D_MODEL], bf16, kind="ExternalInput")
    mask_d = nc.dram_tensor("mask", [128, 128], bf16, kind="ExternalInput")
    out_d = nc.dram_tensor("out", [SEQ, D_MODEL], f32, kind="ExternalOutput")
    recip_d = nc.dram_tensor("recip_scratch", [HPG * QC_TILES, QC], f32)

    with tile.TileContext(nc) as tc:
        with (
            tc.tile_pool(name="persist", bufs=1) as persist,
            tc.tile_pool(name="expsb", bufs=6) as expsb,
            tc.tile_pool(name="rbsb", bufs=4) as rbsb,
            tc.tile_pool(name="rtsb", bufs=4) as rtsb,
            tc.tile_pool(name="outsb", bufs=3) as outsb,
            tc.tile_pool(name="ps_ss", bufs=2, space="PSUM") as ps_ss,
            tc.tile_pool(name="ps_z", bufs=2, space="PSUM") as ps_z,
            tc.tile_pool(name="ps_pj", bufs=2, space="PSUM") as ps_pj,
        ):
            # ---- persistent SBUF tiles ----
            xT = [persist.tile([128, SEQ], bf16, tag=f"xT{i}", name=f"xT{i}") for i in range(MT)]
            wq = [persist.tile([128, HPG * D_HEAD], bf16, tag=f"wq{i}", name=f"wq{i}") for i in range(MT)]
            wk = [persist.tile([128, HPG * D_HEAD], bf16, tag=f"wk{i}", name=f"wk{i}") for i in range(MT)]
            wv = [persist.tile([128, HPG * D_HEAD], bf16, tag=f"wv{i}", name=f"wv{i}") for i in range(MT)]
            wo = [persist.tile([128, D_MODEL], bf16, tag=f"wo{i}", name=f"wo{i}") for i in range(NPAIR)]
            # per-head Q with the opposite 64 rows zeroed (K=128 scores
            # matmul; row-tiled K=64 pairs measured serial on HW, so the
            # zero-padded full-contraction form is faster)
            QTz = [persist.tile([128, SEQ], bf16, tag=f"QTz{h}", name=f"QTz{h}") for h in range(HPG)]
            KT = [persist.tile([128, SEQ], bf16, tag=f"KT{p}", name=f"KT{p}") for p in range(NPAIR)]
            zT = [persist.tile([128, SEQ], bf16, tag=f"zT{p}", name=f"zT{p}") for p in range(NPAIR)]
            V = [persist.tile([128, HPG, D_HEAD + 1], bf16, tag=f"V{i}", name=f"V{i}") for i in range(KT_TILES)]
            mask01 = persist.tile([128, 128], bf16, tag="mask01")

            # ---- input DMAs, ordered for fast start ----
            nc.sync.dma_start(out=mask01, in_=mask_d[:, :])
            cs0 = slice(0, QC)
            for i in range(MT):
                nc.sync.dma_start(out=xT[i][:, cs0], in_=xT_d[128 * i : 128 * (i + 1), cs0])
            for i in range(MT):
                nc.sync.dma_start(out=wq[i], in_=wq_d[128 * i : 128 * (i + 1), :])
            for i in range(MT):
                nc.sync.dma_start(out=wk[i], in_=wk_d[128 * i : 128 * (i + 1), :])
            for i in range(MT):
                nc.sync.dma_start(out=wv[i], in_=wv_d[128 * i : 128 * (i + 1), :])
            for c in range(1, QC_TILES):
                cs = slice(QC * c, QC * (c + 1))
                for i in range(MT):
                    nc.sync.dma_start(out=xT[i][:, cs], in_=xT_d[128 * i : 128 * (i + 1), cs])
            for p in range(NPAIR):
                nc.sync.dma_start(out=wo[p], in_=wo_d[128 * p : 128 * (p + 1), :])
            for h in range(HPG):
                r0 = 64 * (h % 2)
                nc.vector.memset(QTz[h][64 - r0 : 128 - r0, :], 0.0)
            # constant-1 denominator column of V (gpsimd: off the DVE queue)
            for kt in range(KT_TILES):
                nc.gpsimd.memset(V[kt][:, :, D_HEAD : D_HEAD + 1], 1.0)
            # dummy exp so the ACT table load (~1.3us) happens during input DMA
            warm = persist.tile([1, 1], f32, tag="warm")
            nc.vector.memset(warm, 0.0)
            nc.scalar.activation(out=warm, in_=warm,
                                 func=mybir.ActivationFunctionType.Exp, scale=1.0)

            # ---- projection groups (each -> two ~0.6us ballast items) ----
            def emit_qk(w, p, c, is_q):
                cols = slice(128 * p, 128 * (p + 1))
                qs = slice(QC * c, QC * (c + 1))
                cell = []

                def first():
                    ps = ps_pj.tile([128, QC], f32, tag="pj", name="psqk")
                    cell.append(ps)
                    for m in range(3):
                        nc.tensor.matmul(ps, lhsT=w[m][:, cols], rhs=xT[m][:, qs],
                                         start=(m == 0), stop=False,
                                         skip_group_check=True)

                def second():
                    ps = cell[0]
                    for m in range(3, MT):
                        nc.tensor.matmul(ps, lhsT=w[m][:, cols], rhs=xT[m][:, qs],
                                         start=False, stop=(m == MT - 1),
                                         skip_group_check=True)
                    if is_q:
                        nc.vector.tensor_copy(QTz[2 * p][0:64, qs], ps[0:64, :])
                        nc.vector.tensor_copy(QTz[2 * p + 1][64:128, qs], ps[64:128, :])
                    else:
                        nc.vector.tensor_copy(KT[p][:, qs], ps)

                return [first, second]

            def emit_v(kt):
                ks = slice(128 * kt, 128 * (kt + 1))
                cell = []

                def first():
                    ps = ps_pj.tile([128, HPG * D_HEAD], f32, tag="pj", name="psv")
                    cell.append(ps)
                    for m in range(3):
                        nc.tensor.matmul(ps, lhsT=xT[m][:, ks], rhs=wv[m],
                                         start=(m == 0), stop=False,
                                         skip_group_check=True)

                def second():
                    ps = cell[0]
                    for m in range(3, MT):
                        nc.tensor.matmul(ps, lhsT=xT[m][:, ks], rhs=wv[m],
                                         start=False, stop=(m == MT - 1),
                                         skip_group_check=True)
                    nc.vector.tensor_copy(
                        V[kt][:, :, 0:D_HEAD],
                        ps.rearrange("p (h d) -> p h d", h=HPG))

                return [first, second]

            def proj_chunk_units(c):
                units = []
                for p in range(NPAIR):
                    units += emit_qk(wq, p, c, True)
                    units += emit_qk(wk, p, c, False)
                for kt in range(4 * c, 4 * (c + 1)):
                    units += emit_v(kt)
                return units

            # ---- output projection (per 128-row tile of the output) ----
            def outproj_ctile(c):
                cs = slice(128 * c, 128 * (c + 1))
                cell = []

                def mk(p):
                    def fn():
                        if p == 0:
                            cell.append(ps_pj.tile([128, QC], f32, tag="pj", name="pso_a"))
                            cell.append(ps_pj.tile([128, D_MODEL - QC], f32, tag="pj", name="pso_b"))
                        pso_a, pso_b = cell
                        nc.tensor.matmul(pso_a, lhsT=zT[p][:, cs], rhs=wo[p][:, 0:QC],
                                         start=(p == 0), stop=(p == NPAIR - 1),
                                         skip_group_check=True)
                        nc.tensor.matmul(pso_b, lhsT=zT[p][:, cs], rhs=wo[p][:, QC:D_MODEL],
                                         start=(p == 0), stop=(p == NPAIR - 1),
                                         skip_group_check=True)
                    return fn

                def fin():
                    pso_a, pso_b = cell
                    outt = outsb.tile([128, D_MODEL], f32, tag="out", name="outt")
                    nc.vector.tensor_copy(outt[:, 0:QC], pso_a)
                    nc.vector.tensor_copy(outt[:, QC:D_MODEL], pso_b)
                    nc.sync.dma_start(out=out_d[cs, :], in_=outt)

                return [mk(0), mk(1), mk(2), fin]

            def outproj_units(j):
                units = []
                for c in range(4 * j, 4 * (j + 1)):
                    units += outproj_ctile(c)
                return units

            # ---- attention ----
            def emit_scores_pair(p, j, kt2):
                ssA = ps_ss.tile([128, 2 * QC], f32, tag="ss", name="ssA")
                ssB = ps_ss.tile([128, 2 * QC], f32, tag="ss", name="ssB")
                off0 = 0
                for u in (0, 1):
                    kt = kt2 + u
                    delta = kt - 4 * j
                    off = 128 * delta if delta >= 0 else 0
                    if u == 0:
                        off0 = off
                    ks = slice(128 * kt, 128 * (kt + 1))
                    qs = slice(QC * j + off, QC * (j + 1))
                    for h, ss in ((2 * p, ssA), (2 * p + 1, ssB)):
                        nc.tensor.matmul(
                            ss[:, QC * u + off : QC * (u + 1)],
                            lhsT=KT[p][:, ks],
                            rhs=QTz[h][:, qs],
                            start=True, stop=True,
                            skip_group_check=True,
                        )
                expA = expsb.tile([128, 2 * QC], bf16, tag="exp", name="expA")
                expB = expsb.tile([128, 2 * QC], bf16, tag="exp", name="expB")
                for ss, ex in ((ssA, expA), (ssB, expB)):
                    nc.scalar.activation(out=ex[:, off0:], in_=ss[:, off0:],
                                         func=mybir.ActivationFunctionType.Exp,
                                         scale=0.125)
                for u in (0, 1):
                    delta = kt2 + u - 4 * j
                    if delta >= 0:
                        blk = slice(QC * u + 128 * delta, QC * u + 128 * delta + 128)
                        nc.vector.tensor_mul(expA[:, blk], expA[:, blk], mask01)
                        nc.vector.tensor_mul(expB[:, blk], expB[:, blk], mask01)
                return expA, expB

            def emit_pv(p, j, kt2, expA, expB, zA, zB, nkt):
                for u in (0, 1):
                    kt = kt2 + u
                    delta = kt - 4 * j
                    off = 128 * delta if delta >= 0 else 0
                    for h, ex, z in ((2 * p, expA, zA), (2 * p + 1, expB, zB)):
                        nc.tensor.matmul(
                            z[:, off:QC],
                            lhsT=V[kt][:, h, :],
                            rhs=ex[:, QC * u + off : QC * (u + 1)],
                            start=(kt == 0), stop=(kt == nkt - 1),
                            skip_group_check=True,
                        )

            def emit_norm(p, j, zA, zB):
                qs = slice(QC * j, QC * (j + 1))
                for h, psz, r0 in ((2 * p, zA, 0), (2 * p + 1, zB, 64)):
                    row = HPG * j + (h % HPG)
                    # reciprocal_approx_* is a custom DVE op: PSUM input reads
                    # garbage (verified on HW), so stage the denominator row
                    # through SBUF first.
                    dtmp = rtsb.tile([1, QC], f32, tag="dt", name="dtmp")
                    nc.vector.tensor_copy(dtmp, psz[D_HEAD : D_HEAD + 1, :])
                    rtmp = rtsb.tile([1, QC], f32, tag="rt", name="rtmp")
                    nc.vector.reciprocal_approx_fast(rtmp, dtmp)
                    nc.sync.dma_start(out=recip_d[row : row + 1, :], in_=rtmp)
                    nc.vector.tensor_copy(zT[p][r0 : r0 + 64, qs], psz[0:D_HEAD, :])
                    sl = recip_d[row : row + 1, :]
                    rb = rbsb.tile([128, QC], f32, tag="rb", name="rb")
                    nc.sync.dma_start(
                        out=rb[r0 : r0 + 64, :],
                        in_=bass.AP(tensor=sl.tensor, offset=sl.offset,
                                    ap=[[0, D_HEAD]] + list(sl.ap[-1:])))
                    nc.vector.tensor_mul(zT[p][r0 : r0 + 64, qs],
                                         zT[p][r0 : r0 + 64, qs],
                                         rb[r0 : r0 + 64, :])

            def attention_chunk(j, units):
                # proportional pacing: spread the chunk's ballast units
                # (projections for chunk j+1 / deferred output projections)
                # evenly over the chunk's attention steps so the PE never
                # starves while ACT works through the exps.
                nkt = 4 * j + 4
                steps_total = NPAIR * (nkt // 2 + 1)
                state = [0, 0]  # emitted, step

                def pace():
                    state[1] += 1
                    target = len(units) * state[1] // steps_total
                    while state[0] < target:
                        units[state[0]]()
                        state[0] += 1

                for p in range(NPAIR):
                    zA = ps_z.tile([D_HEAD + 1, QC], f32, tag="z", name="zA")
                    zB = ps_z.tile([D_HEAD + 1, QC], f32, tag="z", name="zB")
                    pend = None
                    for kt2 in range(0, nkt, 2):
                        cur = (kt2, emit_scores_pair(p, j, kt2))
                        pace()
                        if pend is not None:
                            kt2p, (eA, eB) = pend
                            emit_pv(p, j, kt2p, eA, eB, zA, zB, nkt)
                        pend = cur
                    pace()
                    kt2p, (eA, eB) = pend
                    emit_pv(p, j, kt2p, eA, eB, zA, zB, nkt)
                    emit_norm(p, j, zA, zB)
                while state[0] < len(units):
                    units[state[0]]()
                    state[0] += 1

            # ---- main schedule ----
            for fn in proj_chunk_units(0):  # eagerly; PE chases the DMAs
                fn()
            chunk_units = {
                0: proj_chunk_units(1),
                1: proj_chunk_units(2),
                2: proj_chunk_units(3),
                3: outproj_units(0) + outproj_units(1) + outproj_units(2),
            }
            for j in range(QC_TILES):
                attention_chunk(j, chunk_units[j])
            for fn in outproj_units(3):
                fn()

    nc.finalize()
    return nc


def kernel(**inputs):
    x = inputs["normalized_resid_pre"]
    W_Q, W_K, W_V, W_O = inputs["W_Q"], inputs["W_K"], inputs["W_V"], inputs["W_O"]
    b_Q, b_K, b_V, b_O = inputs["b_Q"], inputs["b_K"], inputs["b_V"], inputs["b_O"]

    expected = (
        x.shape == (BATCH, SEQ, D_MODEL)
        and W_Q.shape == (N_HEADS, D_MODEL, D_HEAD)
        and W_K.shape == (N_HEADS, D_MODEL, D_HEAD)
        and W_V.shape == (N_HEADS, D_MODEL, D_HEAD)
        and W_O.shape == (N_HEADS, D_HEAD, D_MODEL)
        and not np.any(b_Q)
    )
    if not expected:
        return _numpy_ref(**inputs)

    from concourse.bass_utils import run_bass_kernel_spmd

    if "nc" not in _prog_cache:
        _prog_cache["nc"] = _build_program()
    nc = _prog_cache["nc"]

    # host-side prep: transpose + cast + pack per head-group
    xT = np.ascontiguousarray(x.transpose(0, 2, 1)).astype(BF16)  # [B, 768, 2048]
    # b_K shifts every score in a softmax row equally -> cancels exactly.
    groups = []
    for g in range(2):
        hs = slice(HPG * g, HPG * (g + 1))
        groups.append({
            "wq": np.ascontiguousarray(W_Q[hs].transpose(1, 0, 2).reshape(D_MODEL, HPG * D_HEAD)).astype(BF16),
            "wk": np.ascontiguousarray(W_K[hs].transpose(1, 0, 2).reshape(D_MODEL, HPG * D_HEAD)).astype(BF16),
            "wv": np.ascontiguousarray(W_V[hs].transpose(1, 0, 2).reshape(D_MODEL, HPG * D_HEAD)).astype(BF16),
            "wo": np.ascontiguousarray(W_O[hs].reshape(HPG * D_HEAD, D_MODEL)).astype(BF16),
        })
    ii, jj = np.arange(128)[:, None], np.arange(128)[None, :]
    mask = np.where(jj >= ii, np.float32(1.0), np.float32(0.0)).astype(BF16)

    in_maps = []
    for c in range(NCORES):
        b, g = c // 2, c % 2
        m = {"xT": xT[b], "mask": mask}
        m.update(groups[g])
        in_maps.append(m)

    trace = bool(os.environ.get("ATTN_KERNEL_TRACE"))
    res = run_bass_kernel_spmd(nc, in_maps, list(range(NCORES)), trace=trace)
    _prog_cache["last_exec_time_ns"] = res.exec_time_ns
    _prog_cache["last_results"] = res

    # b_V/b_O fold into a constant row (softmax weights sum to 1).
    const_row = np.einsum("hd,hdm->m", b_V.astype(np.float64), W_O.astype(np.float64))
    const_row = (const_row + b_O.astype(np.float64)).astype(np.float32)

    out = np.empty((BATCH, SEQ, D_MODEL), dtype=np.float32)
    for b in range(BATCH):
        out[b] = res.results[2 * b]["out"] + res.results[2 * b + 1]["out"] + const_row
    return out


# revision 9
# speedup vs baseline: 1.0848x; 1.0848x over previous
"""Multi-head causal attention (dense_transformer) on 8 trn2 NeuronCores.

Problem: x[4, 2048, 768], 12 heads of d_head=64, causal softmax, out proj.

Sharding: data-parallel over batch (4) x tensor-parallel over heads
(2 groups of 6). Core c handles (batch c//2, heads 6*(c%2)..6*(c%2)+5) and
returns its partial output sum over its heads; the host adds the two
partials per batch ("all-reduce" of size 2 done host-side).

Device kernel layout (everything lives transposed so no on-device
transposes are needed; the host pre-transposes x):
  xT  [768, 2048]  bf16   (host-transposed activation)
  QT/KT = W.T @ xT -> [64, 2048] per head (stored as 3 pair-tiles [128, 2048])
  V = xT.T @ Wv -> [2048, 384] natural (stored per k-tile [128, 6, 65];
      column 65 of each head slot is a constant 1.0 so the PV matmul also
      accumulates the softmax denominator as output row 64)
  scoresT tiles [k=128, q=512] = KT_tile.T @ QT_chunk (PSUM), causal
      handled by narrowing the q-range and a -30000 additive mask matmul
      (identity stationary) on diagonal blocks
  softmax without max-subtraction (scores here are O(1); exp cannot
      overflow): P = exp(s/8) / sum_k exp(s/8)
  z^T unnormalized accumulated over k-tiles in PSUM [65, 512]; row 64 is
      the denominator. Normalization: reciprocal -> K=1 broadcast matmul
      -> elementwise multiply, written to zT bf16.
  out = sum_pairs zT_pair.T @ WO_pair -> [2048, 768] fp32, DMA'd out.

Biases: b_K provably cancels in softmax (it shifts every score in a row
by the same amount). b_V and b_O contribute sum_h b_V[h] @ W_O[h] + b_O,
a constant row added host-side. A nonzero b_Q would need a device-side
per-key score offset; inputs here always have b_Q = 0, so that case (and
any unexpected shape) falls back to a numpy reference implementation.
"""
import os
import sys
from collections import deque

sys.path.insert(0, "/opt/trn_rl_repo")

import numpy as np
import ml_dtypes

D_MODEL, N_HEADS, D_HEAD = 768, 12, 64
BATCH, SEQ = 4, 2048
HPG = 6           # heads per group (per core)
NPAIR = HPG // 2  # head pairs per core
NCORES = 8
QC = 512          # q chunk (moving operand width)
KT_TILES = SEQ // 128
QC_TILES = SEQ // QC
MT = D_MODEL // 128  # contraction tiles for projections
BF16 = ml_dtypes.bfloat16

_prog_cache = {}


def _numpy_ref(normalized_resid_pre, W_Q, W_K, W_V, W_O, b_Q, b_K, b_V, b_O):
    x = normalized_resid_pre.astype(np.float32)
    Q = np.einsum("bsm,hmd->bshd", x, W_Q) + b_Q
    K = np.einsum("bsm,hmd->bshd", x, W_K) + b_K
    V = np.einsum("bsm,hmd->bshd", x, W_V) + b_V
    scores = np.einsum("bqhd,bkhd->bhqk", Q, K) / np.sqrt(np.float32(W_Q.shape[-1]))
    s = x.shape[1]
    causal = np.tril(np.ones((s, s), dtype=bool))
    scores = np.where(causal, scores, -np.inf)
    scores -= scores.max(axis=-1, keepdims=True)
    e = np.exp(scores)
    probs = e / e.sum(axis=-1, keepdims=True)
    z = np.einsum("bkhd,bhqk->bqhd", V, probs)
    return (np.einsum("bqhd,hdm->bqm", z, W_O) + b_O).astype(np.float32)


def _build_program():
    from concourse import bacc, tile
    import concourse.bass as bass
    import concourse.mybir as mybir

    f32 = mybir.dt.float32
    bf16 = mybir.dt.bfloat16

    nc = bacc.Bacc(None)
    xT_d = nc.dram_tensor("xT", [D_MODEL, SEQ], bf16, kind="ExternalInput")
    wq_d = nc.dram_tensor("wq", [D_MODEL, HPG * D_HEAD], bf16, kind="ExternalInput")
    wk_d = nc.dram_tensor("wk", [D_MODEL, HPG * D_HEAD], bf16, kind="ExternalInput")
    wv_d = nc.dram_tensor("wv", [D_MODEL, HPG * D_HEAD], bf16, kind="ExternalInput")
    wo_d = nc.dram_tensor("wo", [HPG * D_HEAD, D_MODEL], bf16, kind="ExternalInput")
    mask_d = nc.dram_tensor("mask", [128, 128], bf16, kind="ExternalInput")
    out_d = nc.dram_tensor("out", [SEQ, D_MODEL], f32, kind="ExternalOutput")
    recip_d = nc.dram_tensor("recip_scratch", [HPG * (SEQ // QC), QC], f32)

    with tile.TileContext(nc) as tc:
        with (
            tc.tile_pool(name="persist", bufs=1) as persist,
            tc.tile_pool(name="expsb", bufs=8) as expsb,
            tc.tile_pool(name="rbsb", bufs=4) as rbsb,
            tc.tile_pool(name="outsb", bufs=3) as outsb,
            tc.tile_pool(name="dtmpsb", bufs=8) as dtmpsb,
            tc.tile_pool(name="ps_big", bufs=3, space="PSUM") as ps_big,
            tc.tile_pool(name="ps_z", bufs=2, space="PSUM") as ps_z,
        )    :
            # ---- persistent SBUF tiles ----
            xT = [persist.tile([128, SEQ], bf16, tag=f"xT{i}", name=f"xT{i}") for i in range(MT)]
            wq = [persist.tile([128, HPG * D_HEAD], bf16, tag=f"wq{i}", name=f"wq{i}") for i in range(MT)]
            wk = [persist.tile([128, HPG * D_HEAD], bf16, tag=f"wk{i}", name=f"wk{i}") for i in range(MT)]
            wv = [persist.tile([128, HPG * D_HEAD], bf16, tag=f"wv{i}", name=f"wv{i}") for i in range(MT)]
            wo = [persist.tile([128, D_MODEL], bf16, tag=f"wo{i}", name=f"wo{i}") for i in range(NPAIR)]
            QTz = [persist.tile([128, SEQ], bf16, tag=f"QTz{i}", name=f"QTz{i}") for i in range(HPG)]
            KT = [persist.tile([128, SEQ], bf16, tag=f"KT{i}", name=f"KT{i}") for i in range(NPAIR)]
            zT = [persist.tile([128, SEQ], bf16, tag=f"zT{i}", name=f"zT{i}") for i in range(NPAIR)]
            V = [persist.tile([128, HPG, D_HEAD + 1], bf16, tag=f"V{i}", name=f"V{i}") for i in range(KT_TILES)]
            mask01 = persist.tile([128, 128], bf16, tag="mask01")

            # ---- input DMAs, ordered so the first projection groups can
            # start as soon as wq/wk + x's q-chunk 0 have landed ----
            nc.sync.dma_start(out=mask01, in_=mask_d[:, :])
            for i in range(MT):
                nc.sync.dma_start(out=wq[i], in_=wq_d[128 * i : 128 * (i + 1), :])
                nc.sync.dma_start(out=wk[i], in_=wk_d[128 * i : 128 * (i + 1), :])
            cs0 = slice(0, QC)
            for i in range(MT):
                nc.sync.dma_start(out=xT[i][:, cs0], in_=xT_d[128 * i : 128 * (i + 1), cs0])
            for i in range(MT):
                nc.sync.dma_start(out=wv[i], in_=wv_d[128 * i : 128 * (i + 1), :])
            for c in range(1, QC_TILES):
                cs = slice(QC * c, QC * (c + 1))
                for i in range(MT):
                    nc.sync.dma_start(out=xT[i][:, cs], in_=xT_d[128 * i : 128 * (i + 1), cs])
            for p in range(NPAIR):
                nc.sync.dma_start(out=wo[p], in_=wo_d[128 * p : 128 * (p + 1), :])
            for h in range(HPG):
                r0 = 64 * (h % 2)
                nc.vector.memset(QTz[h][64 - r0 : 128 - r0, :], 0.0)
            for kt in range(KT_TILES):
                nc.vector.memset(V[kt][:, :, D_HEAD : D_HEAD + 1], 1.0)
            # dummy exp so the ACT table load (~2.7us) overlaps input DMAs
            # instead of stalling the first attention exp
            warm = persist.tile([1, 1], f32, tag="warm")
            nc.vector.memset(warm, 0.0)
            nc.scalar.activation(out=warm, in_=warm,
                                 func=mybir.ActivationFunctionType.Exp, scale=1.0)

            # ---- emission helpers ----
            def emit_qkv_pair(p):
                cols = slice(128 * p, 128 * (p + 1))
                for j in range(QC_TILES):
                    qs = slice(QC * j, QC * (j + 1))
                    psq = ps_big.tile([128, QC], f32, tag="big", name="psq")
                    for m in range(MT):
                        nc.tensor.matmul(psq, lhsT=wq[m][:, cols], rhs=xT[m][:, qs],
                                         start=(m == 0), stop=(m == MT - 1))
                    nc.scalar.activation(out=QTz[2 * p][0:64, qs], in_=psq[0:64, :],
                                         func=mybir.ActivationFunctionType.Copy)
                    nc.scalar.activation(out=QTz[2 * p + 1][64:128, qs], in_=psq[64:128, :],
                                         func=mybir.ActivationFunctionType.Copy)
                    psk = ps_big.tile([128, QC], f32, tag="big", name="psk")
                    for m in range(MT):
                        nc.tensor.matmul(psk, lhsT=wk[m][:, cols], rhs=xT[m][:, qs],
                                         start=(m == 0), stop=(m == MT - 1))
                    nc.scalar.activation(out=KT[p][:, qs], in_=psk,
                                         func=mybir.ActivationFunctionType.Copy)

            def emit_v(kts):
                for kt in kts:
                    ks = slice(128 * kt, 128 * (kt + 1))
                    psv = ps_big.tile([128, HPG * D_HEAD], f32, tag="big", name="psv")
                    for m in range(MT):
                        nc.tensor.matmul(psv, lhsT=xT[m][:, ks], rhs=wv[m],
                                         start=(m == 0), stop=(m == MT - 1))
                    nc.vector.tensor_copy(
                        V[kt][:, :, 0:D_HEAD],
                        psv.rearrange("p (h d) -> p h d", h=HPG))

            def emit_scores(h, j, kt2):
                p = h // 2
                pss = ps_big.tile([128, 2 * QC], f32, tag="big", name="pss")
                off0 = 0
                for u in (0, 1):
                    kt = kt2 + u
                    delta = kt - 4 * j  # >=0 on diagonal blocks
                    off = 128 * delta if delta >= 0 else 0
                    if u == 0:
                        off0 = off
                    nc.tensor.matmul(
                        pss[:, QC * u + off : QC * (u + 1)],
                        lhsT=KT[p][:, 128 * kt : 128 * (kt + 1)],
                        rhs=QTz[h][:, QC * j + off : QC * (j + 1)],
                        start=True, stop=True,
                        skip_group_check=True,
                    )
                expt = expsb.tile([128, 2 * QC], bf16, tag="exp", name="expt")
                nc.scalar.activation(out=expt[:, off0:], in_=pss[:, off0:],
                                     func=mybir.ActivationFunctionType.Exp,
                                     scale=0.125)
                for u in (0, 1):
                    delta = kt2 + u - 4 * j
                    if delta >= 0:
                        off = 128 * delta
                        blk = slice(QC * u + off, QC * u + off + 128)
                        nc.vector.tensor_mul(expt[:, blk], expt[:, blk], mask01)
                return expt

            def emit_pv(h, j, psz, nkt, kt2, expt):
                for u in (0, 1):
                    kt = kt2 + u
                    delta = kt - 4 * j
                    off = 128 * delta if delta >= 0 else 0
                    nc.tensor.matmul(
                        psz[:, off:QC],
                        lhsT=V[kt][:, h, :],
                        rhs=expt[:, QC * u + off : QC * (u + 1)],
                        start=(kt == 0), stop=(kt == nkt - 1),
                        skip_group_check=True,
                    )

            def emit_norm(h, j, psz):
                # per-head normalization chain (approx reciprocal -> DRAM
                # hop -> partition-broadcast DMA -> multiply); hides behind
                # subsequent attention work
                p, r0 = h // 2, 64 * (h % 2)
                qs = slice(QC * j, QC * (j + 1))
                row = HPG * j + h
                dtmp = dtmpsb.tile([1, QC], f32, tag="dtmp", name="dtmp")
                nc.vector.tensor_copy(dtmp, psz[D_HEAD : D_HEAD + 1, :])
                rtmp = dtmpsb.tile([1, QC], f32, tag="rtmp", name="rtmp")
                nc.vector.reciprocal_approx_fast(rtmp, dtmp)
                nc.sync.dma_start(out=recip_d[row : row + 1, :], in_=rtmp)
                nc.vector.tensor_copy(zT[p][r0 : r0 + 64, qs], psz[0:D_HEAD, :])
                sl = recip_d[row : row + 1, :]
                rb = rbsb.tile([128, QC], f32, tag="rb", name="rb")
                nc.sync.dma_start(
                    out=rb[r0 : r0 + 64, :],
                    in_=bass.AP(tensor=sl.tensor, offset=sl.offset,
                                ap=[[0, D_HEAD]] + list(sl.ap[-1:])))
                nc.vector.tensor_mul(zT[p][r0 : r0 + 64, qs],
                                     zT[p][r0 : r0 + 64, qs],
                                     rb[r0 : r0 + 64, :])

            def emit_attention(h, j, carry):
                # k-loop with scores staggered two k-pairs ahead of PV. The
                # tail PVs + normalization are returned via `carry` as
                # closures and emitted inside the NEXT unit's score stream,
                # so the PE never drains waiting on the freshest exps at a
                # unit boundary (cross-unit software pipelining).
                nkt = 4 * j + 4
                psz = ps_z.tile([D_HEAD + 1, QC], f32, tag="z", name="psz")
                pend = deque()
                for kt2 in range(0, nkt, 2):
                    expt = emit_scores(h, j, kt2)
                    pend.append((kt2, expt))
                    if carry:
                        carry.popleft()()
                    elif len(pend) > 2:
                        kt2p, exptp = pend.popleft()
                        emit_pv(h, j, psz, nkt, kt2p, exptp)

                def mk_pv(kt2p, exptp):
                    return lambda: emit_pv(h, j, psz, nkt, kt2p, exptp)

                while pend:
                    carry.append(mk_pv(*pend.popleft()))
                carry.append(lambda: emit_norm(h, j, psz))

            def emit_outproj_ctile(c):
                    cs = slice(128 * c, 128 * (c + 1))
                    pso = ps_big.tile([128, D_MODEL], f32, tag="big", name="pso")
                    for p in range(NPAIR):
                        nc.tensor.matmul(pso[:, 0:512], lhsT=zT[p][:, cs],
                                         rhs=wo[p][:, 0:512],
                                         start=(p == 0), stop=(p == NPAIR - 1))
                        nc.tensor.matmul(pso[:, 512:768], lhsT=zT[p][:, cs],
                                         rhs=wo[p][:, 512:768],
                                         start=(p == 0), stop=(p == NPAIR - 1))
                    outt = outsb.tile([128, D_MODEL], f32, tag="out", name="outt")
                    nc.vector.tensor_copy(outt, pso)
                    nc.sync.dma_start(out=out_d[cs, :], in_=outt)

            def emit_outproj(j):
                for c in range(4 * j, 4 * (j + 1)):
                    emit_outproj_ctile(c)

            # ---- schedule: j=0 attention interleaves into the projection
            # phase (PE issues in order; exp latency of the small j=0 blocks
            # hides inside projection matmul streams). outproj(2) runs after
            # the last attention unit so the PE stays warm while the final
            # norm chains' DMA roundtrips complete; outproj(3) follows. ----
            carry = deque()

            def drain_carry():
                while carry:
                    carry.popleft()()

            emit_qkv_pair(0)
            emit_v([0, 1, 2, 3])
            emit_attention(0, 0, carry)
            emit_attention(1, 0, carry)
            emit_qkv_pair(1)
            emit_v([4, 5, 6, 7])
            drain_carry()
            emit_attention(2, 0, carry)
            emit_attention(3, 0, carry)
            emit_qkv_pair(2)
            emit_v([8, 9, 10, 11])
            drain_carry()
            emit_attention(4, 0, carry)
            emit_attention(5, 0, carry)
            emit_v([12, 13, 14, 15])
            drain_carry()
            for j in range(1, QC_TILES):
                for h in range(HPG):
                    emit_attention(h, j, carry)
                    if h == 1 and j < QC_TILES - 1:
                        emit_outproj(j - 1)
            for c in range(4 * (QC_TILES - 2), 4 * (QC_TILES - 1)):
                emit_outproj_ctile(c)
                if carry:
                    carry.popleft()()
            drain_carry()
            emit_outproj(QC_TILES - 1)

    nc.finalize()
    return nc


def kernel(**inputs):
    x = inputs["normalized_resid_pre"]
    W_Q, W_K, W_V, W_O = inputs["W_Q"], inputs["W_K"], inputs["W_V"], inputs["W_O"]
    b_Q, b_K, b_V, b_O = inputs["b_Q"], inputs["b_K"], inputs["b_V"], inputs["b_O"]

    expected = (
        x.shape == (BATCH, SEQ, D_MODEL)
        and W_Q.shape == (N_HEADS, D_MODEL, D_HEAD)
        and W_K.shape == (N_HEADS, D_MODEL, D_HEAD)
        and W_V.shape == (N_HEADS, D_MODEL, D_HEAD)
        and W_O.shape == (N_HEADS, D_HEAD, D_MODEL)
        and not np.any(b_Q)
    )
    if not expected:
        return _numpy_ref(**inputs)

    from concourse.bass_utils import run_bass_kernel_spmd

    if "nc" not in _prog_cache:
        _prog_cache["nc"] = _build_program()
    nc = _prog_cache["nc"]

    # host-side prep: transpose + cast + pack per head-group
    xT = np.ascontiguousarray(x.transpose(0, 2, 1)).astype(BF16)  # [B, 768, 2048]
    # b_K shifts every score in a softmax row equally -> cancels exactly.
    groups = []
    for g in range(2):
        hs = slice(HPG * g, HPG * (g + 1))
        groups.append({
            "wq": np.ascontiguousarray(W_Q[hs].transpose(1, 0, 2).reshape(D_MODEL, HPG * D_HEAD)).astype(BF16),
            "wk": np.ascontiguousarray(W_K[hs].transpose(1, 0, 2).reshape(D_MODEL, HPG * D_HEAD)).astype(BF16),
            "wv": np.ascontiguousarray(W_V[hs].transpose(1, 0, 2).reshape(D_MODEL, HPG * D_HEAD)).astype(BF16),
            "wo": np.ascontiguousarray(W_O[hs].reshape(HPG * D_HEAD, D_MODEL)).astype(BF16),
        })
    ii, jj = np.arange(128)[:, None], np.arange(128)[None, :]
    mask = np.where(jj >= ii, np.float32(1.0), np.float32(0.0)).astype(BF16)

    in_maps = []
    for c in range(NCORES):
        b, g = c // 2, c % 2
        m = {"xT": xT[b], "mask": mask}
        m.update(groups[g])
        in_maps.append(m)

    trace = bool(os.environ.get("ATTN_KERNEL_TRACE"))
    res = run_bass_kernel_spmd(nc, in_maps, list(range(NCORES)), trace=trace)
    _prog_cache["last_exec_time_ns"] = res.exec_time_ns
    _prog_cache["last_results"] = res

    # b_V/b_O fold into a constant row (softmax weights sum to 1).
    const_row = np.einsum("hd,hdm->m", b_V.astype(np.float64), W_O.astype(np.float64))
    const_row = (const_row + b_O.astype(np.float64)).astype(np.float32)

    out = np.empty((BATCH, SEQ, D_MODEL), dtype=np.float32)
    for b in range(BATCH):
        out[b] = res.results[2 * b]["out"] + res.results[2 * b + 1]["out"] + const_row
    return out

